# revision 9
# baseline (speedup 1.0000x reference)
"""Cross-attention (global, batch-flattened K/V) Trainium2 kernel.

Problem: emb [16, 4096, 64]; two cross-attention halves:
  out_l2u = cross(q=emb[:8],  kv=emb[8:])   -> rows 0..7
  out_u2l = cross(q=emb[8:],  kv=emb[:8])   -> rows 8..15
cross(): q/k/v proj (64->512), s = einsum('bnc,nd->bcd', q, kflat),
InstanceNorm over (CH, B*CH) plane per b, softmax over d, ctx = a @ vflat^T,
out = ctx @ Wout.

Sharding: 16 independent (cross, q-batch) instances, 2 per core.
Cores 0-3: q from lower half (kv = upper), cores 4-7: q from upper
(kv = lower), so each core needs k/v projections of one half only.
No collectives; weights replicated.

Per-core dataflow (all matmuls in float32r, 1 cycle/row on the PE):
  phase A: vT[b'] = (kv[b'] @ Wv)^T via PE -> DRAM scratch [8, 512, 4096]
  per instance:
    q = emb_q @ Wq resident in SBUF (lhsT layout via PE transposes)
    s[c, d] accumulated in PSUM over n; k-chunks projected on the fly
      (kf never touches DRAM); stats (sum, sumsq) fused on PSUM drain
    InstanceNorm + exp fused into one ACT pass (scale/bias per partition),
      row-sums via accum_out; softmax division deferred to ctx output
    aT via PE transposes
    ctxT[c, n] accumulated in PSUM over d, vT streamed from DRAM
    out = ctxT^T @ Wout via PE, DMA to output
"""

import numpy as np
import concourse.bass as bass
import concourse.mybir as mybir
import concourse.tile as tile
from concourse import bacc
from concourse.bass_utils import run_bass_kernel_spmd

dt = mybir.dt
AF = mybir.ActivationFunctionType
ALU = mybir.AluOpType

B = 8            # batches per half
N = 4096         # sequence length
C = 64           # embedding channels
CH = 512         # num_heads * C
NB = N // 128    # 32 n-blocks
NCH = N // 512   # 8 chunks of 512
CB = CH // 128   # 4 c-blocks
D = B * CH       # 4096 flattened kv dim
EPS = 1e-5
MM = dt.float32r  # matmul operand dtype
PLANE = float(CH * D)  # InstanceNorm plane size per instance

_nc = None


def _build():
    nc = bacc.Bacc("TRN2", target_bir_lowering=False, debug=False, num_devices=8)

    embq = nc.declare_dram_parameter("embq", [2, N, C], dt.float32, isOutput=False)
    embkv = nc.declare_dram_parameter("embkv", [B, N, C], dt.float32, isOutput=False)
    Wq_d = nc.declare_dram_parameter("Wq", [C, CH], dt.float32, isOutput=False)
    Wk_d = nc.declare_dram_parameter("Wk", [C, CH], dt.float32, isOutput=False)
    Wv_d = nc.declare_dram_parameter("Wv", [C, CH], dt.float32, isOutput=False)
    Wout_d = nc.declare_dram_parameter("Wout", [CH, C], dt.float32, isOutput=False)
    ident_d = nc.declare_dram_parameter("ident", [128, 128], dt.float32, isOutput=False)
    ones_d = nc.declare_dram_parameter("ones", [128, 128], dt.float32, isOutput=False)
    out_d = nc.declare_dram_parameter("out", [2, N, C], dt.float32, isOutput=True)

    vT_dram = nc.dram_tensor("vT_scratch", [B, CH, N], MM)

    with tile.TileContext(nc) as tc:
        with (
            tc.tile_pool(name="const", bufs=1) as constp,
            tc.tile_pool(name="io", bufs=2) as iop,
            tc.tile_pool(name="embt", bufs=1) as embtp,
            tc.tile_pool(name="stream", bufs=4) as streamp,
            tc.tile_pool(name="big", bufs=2) as bigp,
            tc.tile_pool(name="small", bufs=2) as smallp,
            tc.tile_pool(name="ps", bufs=8, space="PSUM") as psp,
        ):
            # ---- constants ----
            ident = constp.tile([128, 128], dt.float32, tag="ident")
            nc.sync.dma_start(ident[:], ident_d[:])
            ones_f = iop.tile([128, 128], dt.float32, tag="wst")
            nc.sync.dma_start(ones_f[:], ones_d[:])
            ones_r = constp.tile([128, 128], MM, tag="ones_r")
            nc.vector.tensor_copy(out=ones_r[:], in_=ones_f[:])

            w_rs = {}
            for name, wd in (("Wq", Wq_d), ("Wk", Wk_d), ("Wv", Wv_d)):
                wst = iop.tile([C, CH], dt.float32, tag="wst")
                nc.sync.dma_start(wst[:], wd[:])
                wr = constp.tile([C, CH], MM, tag=f"{name}_r")
                nc.vector.tensor_copy(out=wr[:], in_=wst[:])
                w_rs[name] = wr
            Wq_r, Wk_r, Wv_r = w_rs["Wq"], w_rs["Wk"], w_rs["Wv"]

            wost = iop.tile([128, CB, C], dt.float32, tag="wst")
            nc.sync.dma_start(
                wost[:], Wout_d[:].rearrange("(cb p) c -> p cb c", p=128)
            )
            Wout_r = constp.tile([128, CB, C], MM, tag="Wout_r")
            nc.vector.tensor_copy(out=Wout_r[:], in_=wost[:])

            # ---- helper: build embT [64, N] (f32r) for one batch ----
            def build_embT(src):  # src: DRAM AP [N, C] fp32
                et = embtp.tile([C, N], MM, tag="embT")
                for h in range(2):
                    lt = iop.tile([128, NB // 2, C], dt.float32, tag="embload")
                    nc.sync.dma_start(
                        lt[:],
                        src[h * (N // 2):(h + 1) * (N // 2), :].rearrange(
                            "(nb p) c -> p nb c", p=128
                        ),
                    )
                    for g in range(4):  # 4 transpose groups of 4 n-blocks
                        pt = psp.tile([128, 512], dt.float32, tag="pp")
                        for j in range(4):
                            nc.tensor.transpose(
                                pt[0:C, j * 128:(j + 1) * 128],
                                lt[:, g * 4 + j, :],
                                ident[:],
                            )
                        base = (h * 16 + g * 4) * 128
                        nc.vector.tensor_copy(
                            out=et[:, base:base + 512], in_=pt[0:C, :]
                        )
                return et

            # ---- phase A: vT for all kv batches -> DRAM ----
            for b in range(B):
                et = build_embT(embkv[b])
                for cb in range(CB):
                    for g in range(NCH):
                        pt = psp.tile([128, 512], dt.float32, tag="pp")
                        nc.tensor.matmul(
                            pt[:],
                            Wv_r[:, cb * 128:(cb + 1) * 128],
                            et[:, g * 512:(g + 1) * 512],
                            start=True,
                            stop=True,
                        )
                        st = streamp.tile([128, 512], MM, tag="vst")
                        nc.vector.tensor_copy(out=st[:], in_=pt[:])
                        nc.sync.dma_start(
                            vT_dram[b, cb * 128:(cb + 1) * 128,
                                    g * 512:(g + 1) * 512],
                            st[:],
                        )

            # ---- per instance ----
            for inst in range(2):
                # q resident: [128, nb, ch] f32r
                et_q = build_embT(embq[inst])
                q_sb = bigp.tile([128, NB, CH], MM, tag="big")
                for nb in range(NB):
                    pt = psp.tile([128, 512], dt.float32, tag="pp")
                    nc.tensor.matmul(
                        pt[:],
                        et_q[:, nb * 128:(nb + 1) * 128],
                        Wq_r[:],
                        start=True,
                        stop=True,
                    )
                    nc.vector.tensor_copy(out=q_sb[:, nb, :], in_=pt[:])

                # s = q^T @ kflat, accumulated over n; k projected on the fly
                s_sb = bigp.tile([128, CB, N], dt.float32, tag="big")
                ssum = smallp.tile([128, CB, B], dt.float32, tag="ssum")
                ssq = smallp.tile([128, CB, B], dt.float32, tag="ssq")
                for db in range(B):
                    et = build_embT(embkv[db])
                    ps_s = [psp.tile([128, 512], dt.float32, tag="pp",
                                     name=f"ps_s{cb_}")
                            for cb_ in range(CB)]
                    for nb in range(NB):
                        ptk = psp.tile([128, 512], dt.float32, tag="pp")
                        nc.tensor.matmul(
                            ptk[:],
                            et[:, nb * 128:(nb + 1) * 128],
                            Wk_r[:],
                            start=True,
                            stop=True,
                        )
                        kf = streamp.tile([128, 512], MM, tag="kf")
                        nc.vector.tensor_copy(out=kf[:], in_=ptk[:])
                        for cb in range(CB):
                            nc.tensor.matmul(
                                ps_s[cb][:],
                                q_sb[:, nb, cb * 128:(cb + 1) * 128],
                                kf[:],
                                start=(nb == 0),
                                stop=(nb == NB - 1),
                            )
                    for cb in range(CB):
                        nc.scalar.activation(
                            s_sb[:, cb, db * 512:(db + 1) * 512],
                            ps_s[cb][:],
                            AF.Copy,
                            accum_out=ssum[:, cb, db:db + 1],
                        )
                        scr = smallp.tile([128, 512], dt.float32, tag="scratch")
                        nc.scalar.activation(
                            scr[:],
                            ps_s[cb][:],
                            AF.Square,
                            accum_out=ssq[:, cb, db:db + 1],
                        )

                # ---- InstanceNorm stats -> per-partition scale/bias ----
                red = smallp.tile([128, 2], dt.float32, tag="red")
                nc.vector.tensor_reduce(
                    out=red[:, 0:1], in_=ssum[:], axis=mybir.AxisListType.XY,
                    op=ALU.add,
                )
                nc.vector.tensor_reduce(
                    out=red[:, 1:2], in_=ssq[:], axis=mybir.AxisListType.XY,
                    op=ALU.add,
                )
                red_r = smallp.tile([128, 2], MM, tag="red_r")
                nc.vector.tensor_copy(out=red_r[:], in_=red[:])
                ptr = psp.tile([128, 512], dt.float32, tag="pp")
                # all-partition totals via ones matmul
                nc.tensor.matmul(
                    ptr[:, 0:2], ones_r[:], red_r[:], start=True, stop=True
                )
                stats = smallp.tile([128, 8], dt.float32, tag="stats")
                # mu = tot_sum / PLANE ; ex2 = tot_sq / PLANE
                nc.scalar.activation(
                    stats[:, 0:2], ptr[:, 0:2], AF.Copy, bias=0.0,
                    scale=1.0 / PLANE,
                )
                mu = stats[:, 0:1]
                ex2 = stats[:, 1:2]
                musq = stats[:, 2:3]
                var = stats[:, 3:4]
                std = stats[:, 4:5]
                rstd = stats[:, 5:6]
                nmr = stats[:, 6:7]
                nc.vector.tensor_tensor(out=musq, in0=mu, in1=mu, op=ALU.mult)
                nc.vector.tensor_tensor(out=var, in0=ex2, in1=musq,
                                        op=ALU.subtract)
                nc.vector.tensor_scalar_add(var, var, EPS)
                nc.scalar.activation(std, var, AF.Sqrt, bias=0.0)
                nc.vector.reciprocal(rstd, std)
                nc.vector.tensor_tensor(out=nmr, in0=mu, in1=rstd, op=ALU.mult)
                nc.scalar.mul(nmr, nmr, -1.0)

                # ---- softmax numerator: a = exp((s - mu) * rstd), in place ----
                den = smallp.tile([128, CB], dt.float32, tag="den")
                for cb in range(CB):
                    nc.scalar.activation(
                        s_sb[:, cb, :],
                        s_sb[:, cb, :],
                        AF.Exp,
                        bias=nmr,
                        scale=rstd,
                        accum_out=den[:, cb:cb + 1],
                    )
                inv_den = smallp.tile([128, CB], dt.float32, tag="invden")
                nc.vector.reciprocal(inv_den[:], den[:])

                # ---- aT via PE transposes ----
                aT = bigp.tile([128, NB, CH], MM, tag="big")
                for ds in range(NB):
                    pt = psp.tile([128, 512], dt.float32, tag="pp")
                    for cb in range(CB):
                        nc.tensor.transpose(
                            pt[:, cb * 128:(cb + 1) * 128],
                            s_sb[:, cb, ds * 128:(ds + 1) * 128],
                            ident[:],
                        )
                    nc.vector.tensor_copy(out=aT[:, ds, :], in_=pt[:])

                # ---- ctxT = (a @ vflat^T) / den ----
                ctxT = bigp.tile([128, CB, N], MM, tag="big")
                for g in range(NCH):
                    ps_c = [psp.tile([128, 512], dt.float32, tag="pp",
                                     name=f"ps_c{cb_}")
                            for cb_ in range(CB)]
                    for ds in range(NB):
                        vf = streamp.tile([128, 512], MM, tag="vf")
                        nc.sync.dma_start(
                            vf[:],
                            vT_dram[ds // 4, (ds % 4) * 128:(ds % 4 + 1) * 128,
                                    g * 512:(g + 1) * 512],
                        )
                        for cb in range(CB):
                            nc.tensor.matmul(
                                ps_c[cb][:],
                                aT[:, ds, cb * 128:(cb + 1) * 128],
                                vf[:],
                                start=(ds == 0),
                                stop=(ds == NB - 1),
                            )
                    for cb in range(CB):
                        nc.scalar.activation(
                            ctxT[:, cb, g * 512:(g + 1) * 512],
                            ps_c[cb][:],
                            AF.Copy,
                            scale=inv_den[:, cb:cb + 1],
                        )

                # ---- out = ctx @ Wout ----
                for g in range(4):
                    ot = streamp.tile([128, 8, C], dt.float32, tag="ot")
                    for j in range(8):
                        nb = g * 8 + j
                        po = psp.tile([128, 512], dt.float32, tag="pp")
                        for cb in range(CB):
                            nc.tensor.matmul(
                                po[:, 0:C],
                                ctxT[:, cb, nb * 128:(nb + 1) * 128],
                                Wout_r[:, cb, :],
                                start=(cb == 0),
                                stop=(cb == CB - 1),
                            )
                        nc.vector.tensor_copy(out=ot[:, j, :], in_=po[:, 0:C])
                    nc.sync.dma_start(
                        out_d[inst, g * 1024:(g + 1) * 1024, :].rearrange(
                            "(j p) c -> p j c", p=128
                        ),
                        ot[:],
                    )

    nc.compile()
    return nc


def _get_nc():
    global _nc
    if _nc is None:
        _nc = _build()
    return _nc


def kernel(emb, Wq, Wk, Wv, Wout):
    emb = np.ascontiguousarray(emb, dtype=np.float32)
    Wq = np.ascontiguousarray(Wq, dtype=np.float32)
    Wk = np.ascontiguousarray(Wk, dtype=np.float32)
    Wv = np.ascontiguousarray(Wv, dtype=np.float32)
    Wout = np.ascontiguousarray(Wout, dtype=np.float32)
    emb_l, emb_u = emb[:B], emb[B:]
    ident = np.eye(128, dtype=np.float32)
    ones = np.ones((128, 128), dtype=np.float32)

    in_maps = []
    for core in range(8):
        if core < 4:
            qb, kvb = emb_l[2 * core:2 * core + 2], emb_u
        else:
            j = core - 4
            qb, kvb = emb_u[2 * j:2 * j + 2], emb_l
        in_maps.append({
            "embq": np.ascontiguousarray(qb), "embkv": np.ascontiguousarray(kvb),
            "Wq": Wq, "Wk": Wk, "Wv": Wv, "Wout": Wout, "ident": ident,
            "ones": ones,
        })

    res = run_bass_kernel_spmd(_get_nc(), in_maps, list(range(8))).results

    out = np.empty((2 * B, N, C), np.float32)
    for core in range(8):
        if core < 4:
            out[2 * core:2 * core + 2] = res[core]["out"]
        else:
            j = core - 4
            out[B + 2 * j:B + 2 * j + 2] = res[core]["out"]
    return out
